# revision 1
# baseline (speedup 1.0000x reference)
"""Trainium2 Bass kernel for nn_MCPBRNN_GWVariant_Routing.

The reference flattens x [B,S] (b-major) into one 4.2M-step sequential scan
with scalar state:

    oo_t = oo1 * sigmoid(be + c_t * w1e)
    c_{t+1} = (1 - oo_t) * c_t + x_t

and keeps, per row b, only the values at that row's last step (t = S-1):
h = oo*c, c, oo, f = 1-oo, each placed into a [B,1] output (rows < time_lag
stay zero).

Two observations make this fast on Trainium:

1. The recurrence is strongly contracting (|dc'/dc| ~ 0.5 for any weights
   this model can produce), so the carry at a row's last step depends only on
   the last W inputs before it — which, for W <= S-1, all lie within that
   same row. Each row therefore becomes an independent W-step warmup scan
   from an arbitrary initial guess; the windowing error ~0.55^W is far below
   f32 resolution for W = 64.

2. The W-step nonlinear scan is solved by Picard iteration on the carry
   trajectory: given trajectory estimate C, compute the multipliers
   F = 1 - oo1*sigmoid(be + w1e*C) elementwise (parallel), then rebuild C
   with the hardware affine-scan instruction (tensor_tensor_scan:
   state = F[t]*state + U[t]). The iteration contracts by ~0.2/step; K
   iterations reach f32 roundoff. This turns ~W serial steps into ~K serial
   instructions.

Sharding: 2048 rows -> 8 cores x 256 rows. Per core, 256 rows live as 2
windows per partition (rows p and p+128), concatenated along the free dim.
A zero multiplier at each window start resets the scan state to the initial
guess, so one scan instruction handles all 256 windows of a core.
"""

import math

import numpy as np

import concourse.bass as bass  # noqa: F401  (kept for parity with docs)
import concourse.mybir as mybir
from concourse import bacc, bass_utils
from concourse.tile import TileContext

B, S = 2048, 2048
NCORES = 8
RPC = B // NCORES          # rows per core (256)
NW = RPC // 128            # windows per partition (2)
W = 64                     # warmup window length (contraction ~0.55^W)
K = 12                     # Picard iterations (contraction ~0.2^K)
L = W + 1                  # scan positions per window (init + W steps)
NF = NW * L                # free-dim length of the concatenated windows

_FP = mybir.dt.float32
_MULT = mybir.AluOpType.mult
_ADD = mybir.AluOpType.add

_cache: dict = {}


def _build(w1e: float, be: float, oo1: float, cguess: float):
    nc = bacc.Bacc("TRN2", debug=False, num_devices=NCORES)
    xw = nc.dram_tensor("xw", [RPC, W], _FP, kind="ExternalInput")
    out = nc.dram_tensor("out", [4, RPC], _FP, kind="ExternalOutput")

    sig = mybir.ActivationFunctionType.Sigmoid

    with TileContext(nc) as tc:
        with tc.tile_pool(name="p", bufs=1) as pool:
            bias_t = pool.tile([128, 1], _FP)
            U = pool.tile([128, NF], _FP)       # scan addend: [guess, x window]
            C = pool.tile([128, NF], _FP)       # carry trajectory estimate
            F = pool.tile([128, NF + 1], _FP)   # shifted multipliers + guards
            SGf = pool.tile([128, NW], _FP)
            OUT = pool.tile([128, 4 * NW], _FP)

            nc.vector.memset(bias_t[:, :], be)
            nc.vector.memset(C[:, :], cguess)
            nc.vector.memset(U[:, 0:NF:L], cguess)     # window-start init value
            nc.vector.memset(F[:, 0:NF + 1:L], 0.0)    # guard cols: reset scan

            # U[p, w*L+1+i] = xw[w*128+p, i]
            Uw = U[:, :].rearrange("p (w i) -> p w i", i=L)[:, :, 1:]
            nc.sync.dma_start(Uw, xw.ap().rearrange("(w p) i -> p w i", p=128))

            # A[j] (scan multiplier at position j) = F[j]; F[w*L] = 0 resets
            # the state to U[w*L] = guess at each window start. The
            # activation writes f(C[w*L+i]) to F[w*L+1+i], i < W — i.e. the
            # multiplier view is F pre-shifted by one with guards untouched.
            Cin = C[:, :].rearrange("p (w i) -> p w i", i=L)[:, :, 0:W]
            Fw = F[:, 1:NF + 1].rearrange("p (w i) -> p w i", i=L)[:, :, 0:W]
            A = F[:, 0:NF]
            for _ in range(K):
                nc.scalar.activation(Fw, Cin, sig, bias=bias_t[:, :], scale=w1e)
                nc.vector.tensor_scalar(Fw, Fw, -oo1, 1.0, _MULT, _ADD)
                nc.vector.tensor_tensor_scan(C[:, :], A, U[:, :], 0.0, _MULT, _ADD)

            # Outputs from the final carries C[p, w*L + W].
            cfin = C[:, W:NF:L]                      # [128, NW]
            h_v = OUT[:, 0 * NW:1 * NW]
            c_v = OUT[:, 1 * NW:2 * NW]
            oo_v = OUT[:, 2 * NW:3 * NW]
            f_v = OUT[:, 3 * NW:4 * NW]
            nc.scalar.activation(SGf[:, :], cfin, sig, bias=bias_t[:, :], scale=w1e)
            nc.vector.tensor_scalar_mul(oo_v, SGf[:, :], oo1)
            nc.vector.tensor_scalar(f_v, SGf[:, :], -oo1, 1.0, _MULT, _ADD)
            nc.vector.tensor_copy(c_v, cfin)
            nc.vector.tensor_mul(h_v, oo_v, c_v)

            # out[k, w*128+p] = OUT[p, k*NW+w]
            nc.sync.dma_start(
                out.ap().rearrange("k (w p) -> p k w", p=128),
                OUT[:, :].rearrange("p (k w) -> p k w", w=NW),
            )

    nc.compile()
    return nc


def _prepare(x, c_mean, c_std, weight_r_yom, weight_r_yfm, bias_b0_yom,
             weight_b1_yom):
    x = np.ascontiguousarray(np.asarray(x, dtype=np.float32))
    assert x.shape == (B, S), x.shape
    w_yom = float(np.asarray(weight_r_yom).reshape(-1)[0])
    w_yfm = float(np.asarray(weight_r_yfm).reshape(-1)[0])
    b0 = float(np.asarray(bias_b0_yom).reshape(-1)[0])
    w1 = float(np.asarray(weight_b1_yom).reshape(-1)[0])
    mo = float(np.asarray(c_mean).reshape(-1)[0])
    so = float(np.asarray(c_std).reshape(-1)[0])
    oo1 = math.exp(w_yom) / (math.exp(w_yom) + math.exp(w_yfm))
    w1e = w1 / so
    be = b0 - mo * w1 / so

    xw = np.ascontiguousarray(x[:, S - 1 - W:S - 1])
    # Fixed-point of the mean recurrence as the warmup initial guess. Any
    # O(1) guess works (the window + Picard both contract); this just starts
    # closer.
    um = float(xw.mean())
    c = 1.0
    for _ in range(100):
        c = um / max(oo1 / (1.0 + math.exp(-(be + c * w1e))), 1e-6)
        c = min(max(c, 0.0), 1e6)

    key = (round(w1e, 12), round(be, 12), round(oo1, 12), round(c, 6))
    nc = _cache.get(key)
    if nc is None:
        nc = _build(*key)
        _cache[key] = nc
    in_maps = [
        {"xw": np.ascontiguousarray(xw[i * RPC:(i + 1) * RPC])}
        for i in range(NCORES)
    ]
    return nc, in_maps


def _assemble(results, time_lag):
    full = np.concatenate([r["out"] for r in results], axis=1)  # [4, B]
    tl = int(np.asarray(time_lag))
    outs = []
    for k in range(4):
        o = np.zeros((B, 1), np.float32)
        o[tl:, 0] = full[k, tl:]
        outs.append(o)
    return tuple(outs)


def kernel(x, y_obs, c_mean, c_std, Ini_C, weight_r_yom, weight_r_yfm,
           bias_b0_yom, weight_b1_yom, epoch, time_lag):
    nc, in_maps = _prepare(x, c_mean, c_std, weight_r_yom, weight_r_yfm,
                           bias_b0_yom, weight_b1_yom)
    res = bass_utils.run_bass_kernel_spmd(nc, in_maps, core_ids=list(range(NCORES)))
    return _assemble(res.results, time_lag)


def kernel_traced(x, y_obs, c_mean, c_std, Ini_C, weight_r_yom, weight_r_yfm,
                  bias_b0_yom, weight_b1_yom, epoch, time_lag):
    """Like kernel(), but returns (outputs, BassKernelResults) with an NTFF
    trace so a test harness can report HW exec time."""
    nc, in_maps = _prepare(x, c_mean, c_std, weight_r_yom, weight_r_yfm,
                           bias_b0_yom, weight_b1_yom)
    res = bass_utils.run_bass_kernel_spmd(
        nc, in_maps, core_ids=list(range(NCORES)), trace=True
    )
    return _assemble(res.results, time_lag), res


# revision 2
# speedup vs baseline: 1.6242x; 1.6242x over previous
"""Trainium2 Bass kernel for nn_MCPBRNN_GWVariant_Routing.

The reference flattens x [B,S] (b-major) into one 4.2M-step sequential scan
with scalar state:

    oo_t = oo1 * sigmoid(be + c_t * w1e)
    c_{t+1} = (1 - oo_t) * c_t + x_t

and keeps, per row b, only the values at that row's last step (t = S-1):
h = oo*c, c, oo, f = 1-oo, each placed into a [B,1] output (rows < time_lag
stay zero).

Two observations make this fast on Trainium:

1. The recurrence is strongly contracting (|dc'/dc| ~ 0.5 for any weights
   this model can produce), so the carry at a row's last step depends only on
   the last W inputs before it — which, for W <= S-1, all lie within that
   same row. Each row therefore becomes an independent W-step warmup scan
   from an arbitrary O(1) initial state; the windowing error ~0.55^W is far
   below f32 resolution for W = 32.

2. The W-step nonlinear scan is solved by Picard iteration on the carry
   trajectory: given trajectory estimate C, compute the multipliers
   F = 1 - oo1*sigmoid(be + w1e*C) elementwise (parallel), then rebuild C
   with the hardware affine-scan instruction (tensor_tensor_scan:
   state = F[t]*state + U[t]). The iteration contracts by ~0.2/step; the
   first pass uses a constant F (constant initial trajectory), so 10 scans
   reach f32 roundoff. This turns ~W serial steps into ~10 serial
   instruction rounds.

Sharding: 2048 rows -> 8 cores x 256 rows. Per core, 256 rows live as 2
windows per partition (rows p and p+128), concatenated along the free dim.
A zero multiplier at each window start resets the scan state to that
window's first U element (the x value just before the window — a fine
initial guess), so one scan instruction handles all 256 windows of a core.
"""

import math

import numpy as np

import concourse.mybir as mybir
from concourse import bacc, bass_utils
from concourse.tile import TileContext

B, S = 2048, 2048
NCORES = 8
RPC = B // NCORES          # rows per core (256)
NW = RPC // 128            # windows per partition (2)
W = 32                     # warmup window length (contraction ~0.55^W)
NSCANS = 10                # scan passes (pass 0 uses constant multipliers)
L = W + 1                  # scan positions per window (init + W steps)
NF = NW * L                # free-dim length of the concatenated windows

_FP = mybir.dt.float32
_MULT = mybir.AluOpType.mult
_ADD = mybir.AluOpType.add

_cache: dict = {}


def _build(w1e: float, be: float, oo1: float, cguess: float):
    nc = bacc.Bacc("TRN2", debug=False, num_devices=NCORES)
    xw = nc.dram_tensor("xw", [RPC, L], _FP, kind="ExternalInput")
    out = nc.dram_tensor("out", [128, 4 * NW], _FP, kind="ExternalOutput")

    sig = mybir.ActivationFunctionType.Sigmoid
    f0 = 1.0 - oo1 / (1.0 + math.exp(-(be + cguess * w1e)))

    with TileContext(nc) as tc:
        with tc.tile_pool(name="p", bufs=1) as pool:
            bias_t = pool.tile([128, 1], _FP)
            U = pool.tile([128, NF], _FP)       # scan addend: [init, x window]
            C = pool.tile([128, NF], _FP)       # carry trajectory estimate
            F = pool.tile([128, NF + 1], _FP)   # shifted multipliers + guards
            SGf = pool.tile([128, NW], _FP)
            OUT = pool.tile([128, 4 * NW], _FP)

            nc.vector.memset(bias_t[:, :], be)
            # Pass 0 runs with constant multipliers f(cguess); guard columns
            # stay 0 so the scan state resets to U[w*L] at window starts.
            nc.vector.memset(F[:, :], f0)
            nc.vector.memset(F[:, 0:NF + 1:L], 0.0)

            # U[p, w*L + j] = xw[w*128+p, j]  (j=0 is the init element)
            nc.sync.dma_start(
                U[:, :].rearrange("p (w i) -> p w i", i=L),
                xw.ap().rearrange("(w p) i -> p w i", p=128),
            )

            # A[j] (scan multiplier at position j) = F[j]; F[w*L] = 0 resets
            # the state at each window start. The activation writes
            # f(C[w*L+i]) to F[w*L+1+i], i < W — i.e. the multiplier view is
            # F pre-shifted by one with guards untouched.
            Cin = C[:, :].rearrange("p (w i) -> p w i", i=L)[:, :, 0:W]
            Fw = F[:, 1:NF + 1].rearrange("p (w i) -> p w i", i=L)[:, :, 0:W]
            A = F[:, 0:NF]
            for k in range(NSCANS):
                if k > 0:
                    nc.scalar.activation(Fw, Cin, sig, bias=bias_t[:, :],
                                         scale=w1e)
                    nc.vector.tensor_scalar(Fw, Fw, -oo1, 1.0, _MULT, _ADD)
                nc.vector.tensor_tensor_scan(C[:, :], A, U[:, :], 0.0,
                                             _MULT, _ADD)

            # Outputs from the final carries C[p, w*L + W]; OUT free layout
            # is (k, w) with k in [h, c, oo, f].
            cfin = C[:, W:NF:L]                      # [128, NW]
            h_v = OUT[:, 0 * NW:1 * NW]
            c_v = OUT[:, 1 * NW:2 * NW]
            oo_v = OUT[:, 2 * NW:3 * NW]
            f_v = OUT[:, 3 * NW:4 * NW]
            nc.scalar.activation(SGf[:, :], cfin, sig, bias=bias_t[:, :],
                                 scale=w1e)
            nc.vector.tensor_scalar_mul(oo_v, SGf[:, :], oo1)
            nc.vector.tensor_scalar(f_v, SGf[:, :], -oo1, 1.0, _MULT, _ADD)
            nc.vector.tensor_copy(c_v, cfin)
            nc.vector.tensor_mul(h_v, oo_v, c_v)

            nc.sync.dma_start(out.ap(), OUT[:, :])

    nc.compile()
    return nc


def _prepare(x, c_mean, c_std, weight_r_yom, weight_r_yfm, bias_b0_yom,
             weight_b1_yom):
    x = np.asarray(x, dtype=np.float32)
    assert x.shape == (B, S), x.shape
    w_yom = float(np.asarray(weight_r_yom).reshape(-1)[0])
    w_yfm = float(np.asarray(weight_r_yfm).reshape(-1)[0])
    b0 = float(np.asarray(bias_b0_yom).reshape(-1)[0])
    w1 = float(np.asarray(weight_b1_yom).reshape(-1)[0])
    mo = float(np.asarray(c_mean).reshape(-1)[0])
    so = float(np.asarray(c_std).reshape(-1)[0])
    oo1 = math.exp(w_yom) / (math.exp(w_yom) + math.exp(w_yfm))
    w1e = w1 / so
    be = b0 - mo * w1 / so

    # Per-row warmup windows: the W inputs before each row's last step plus
    # one leading element that seeds the scan state at each window start.
    xw = np.ascontiguousarray(x[:, S - 2 - W:S - 1])
    # Fixed point of the mean recurrence: pass-0 multiplier f(cguess).
    um = float(xw.mean())
    c = 1.0
    for _ in range(100):
        c = um / max(oo1 / (1.0 + math.exp(-(be + c * w1e))), 1e-6)
        c = min(max(c, 0.0), 1e6)

    key = (round(w1e, 12), round(be, 12), round(oo1, 12), round(c, 6))
    nc = _cache.get(key)
    if nc is None:
        nc = _build(*key)
        _cache[key] = nc
    in_maps = [
        {"xw": np.ascontiguousarray(xw[i * RPC:(i + 1) * RPC])}
        for i in range(NCORES)
    ]
    return nc, in_maps


def _assemble(results, time_lag):
    # Per-core out[p, k*NW+w] holds output k of row w*128+p.
    parts = [
        np.transpose(r["out"].reshape(128, 4, NW), (1, 2, 0)).reshape(4, RPC)
        for r in results
    ]
    full = np.concatenate(parts, axis=1)  # [4, B]
    tl = int(np.asarray(time_lag))
    outs = []
    for k in range(4):
        o = np.zeros((B, 1), np.float32)
        o[tl:, 0] = full[k, tl:]
        outs.append(o)
    return tuple(outs)


def kernel(x, y_obs, c_mean, c_std, Ini_C, weight_r_yom, weight_r_yfm,
           bias_b0_yom, weight_b1_yom, epoch, time_lag):
    nc, in_maps = _prepare(x, c_mean, c_std, weight_r_yom, weight_r_yfm,
                           bias_b0_yom, weight_b1_yom)
    res = bass_utils.run_bass_kernel_spmd(nc, in_maps,
                                          core_ids=list(range(NCORES)))
    return _assemble(res.results, time_lag)


def kernel_traced(x, y_obs, c_mean, c_std, Ini_C, weight_r_yom, weight_r_yfm,
                  bias_b0_yom, weight_b1_yom, epoch, time_lag):
    """Like kernel(), but returns (outputs, BassKernelResults) with an NTFF
    trace so a test harness can report HW exec time."""
    nc, in_maps = _prepare(x, c_mean, c_std, weight_r_yom, weight_r_yfm,
                           bias_b0_yom, weight_b1_yom)
    res = bass_utils.run_bass_kernel_spmd(
        nc, in_maps, core_ids=list(range(NCORES)), trace=True
    )
    return _assemble(res.results, time_lag), res
